# revision 7
# baseline (speedup 1.0000x reference)
"""DMN4 topk-masking kernel for Trainium2 (8 NeuronCores, Bass/Tile).

Problem: few-shot episodic loss (DMN4). For each (episode b, query q):
  - cosine similarity S[m, g] between 100 query descriptors (m) and
    2500 support descriptors (g = class w * 500 + shot k * 100 + pos p),
    contracting over c=640 channels.
  - per-class max S_max[w, m]; global argmax "nearest[m]"; top-2 class
    diff; mutual-nearest mask; predict[w] = sum_m S_max*mask*2;
    loss = NLL(log_softmax(predict), y), meaned over all b*q.

Sharding: data-parallel over (b, q). 8 cores = 4 episodes x 2 query
halves; each core processes 38 queries of one episode (cores 2k+1
overlap one query which the host drops when gathering).

Implementation notes:
  - Descriptors are L2-normalized on the host, scaled by 16, padded
    from 640 to 768 channels and cast to fp8 e4m3. The PE then runs
    DoubleRow fp8 matmuls (K=256 per pass, 0.5 cycles/column), halving
    tensor-engine time vs fp32r. The 16x16=256 scale is folded back in
    the PSUM->SBUF copy (scale=1/256).
  - S' is staged to SBUF as fp16 (halves DVE scan cost); per-class max,
    global argmax and all comparisons are done on the fp16 copy, which
    keeps the max/find value-matching self-consistent.
  - The mutual-nearest scatter/gather chain is a 100x100 "same-slot"
    comparison matrix: winner[m] = first argmax over m' of
    (nearest[m']==nearest[m]) * diff[m'], mask = winner==m.
  - Glue ops (index casts, top-2 diff, partition broadcast, score and
    mask element-wise ops) run on the otherwise-idle GpSimd engine to
    keep DVE free for the two big scans (reduce + find_index).
"""

import numpy as np
import ml_dtypes

from concourse import bacc, bass, mybir
from concourse.bass_utils import run_bass_kernel_spmd
from concourse.masks import make_identity
from concourse.tile import TileContext

DT = mybir.dt
AF = mybir.ActivationFunctionType
OP = mybir.AluOpType
PM = mybir.MatmulPerfMode

N_WAY = 5
K_SHOT = 5
TEMPERATURE = 2.0
EPS = 1e-8
B, Q, C, HW = 4, 75, 640, 100
MQ = HW            # query descriptors per query image
MS = K_SHOT * HW   # support descriptors per class
NS = N_WAY * MS    # 2500 support descriptors total
NQ = 38            # queries per core (2 cores x 38 covers 75 with 1 overlap)
IN_SCALE = 16.0    # host-side scale before fp8 rounding (avoids denormals)

# fp8 DoubleRow path: channels padded to 768 = 3 pairs of 128
C_PAD = 768
NT = C_PAD // 256  # 3 DoubleRow passes of K=256
MQP = 128          # query descriptors padded (dual-fp8 ldweights alignment)
MSP = 512          # per-class support cols padded (dual-fp8 ifmap alignment)
NEG16 = -60000.0   # below any similarity, representable in fp16


def build_kernel(mode="fp8"):
    """One SPMD program; every core runs the same 38-query episode slice."""
    nc = bacc.Bacc("TRN2", target_bir_lowering=False, debug=False, num_devices=8)

    if mode == "fp8":
        mm_dt = DT.float8e4
        sup_shape = [128, NT, 2, N_WAY, MSP]  # [cp, t, k2, w, s]
        qry_shape = [128, NQ, NT, 2, MQP]     # [cp, q, t, k2, m]
        out_scale = 1.0 / (IN_SCALE * IN_SCALE)
    else:  # fp32r fallback
        mm_dt = DT.float32r
        CC = C // 128
        sup_shape = [128, CC, N_WAY, MS]
        qry_shape = [128, NQ, CC, MQ]
        out_scale = 1.0

    sup_d = nc.declare_dram_parameter(
        "sup", [128, int(np.prod(sup_shape[1:]))], mm_dt, False)
    qry_d = nc.declare_dram_parameter(
        "qry", [128, int(np.prod(qry_shape[1:]))], mm_dt, False)
    oneh_d = nc.declare_dram_parameter("oneh", [1, NQ * N_WAY], DT.float32, False)
    loss_d = nc.declare_dram_parameter("loss", [1, NQ], DT.float32, True)

    with TileContext(nc) as tc:
        with (
            tc.tile_pool(name="const", bufs=1) as const,
            tc.tile_pool(name="sup", bufs=1) as supp,
            tc.tile_pool(name="qin", bufs=1) as qin,
            tc.tile_pool(name="sb", bufs=3) as sbp,
            tc.tile_pool(name="small", bufs=3) as sm,
            tc.tile_pool(name="out", bufs=1) as outp,
            tc.tile_pool(name="ps", bufs=6, space="PSUM") as pps,
            tc.tile_pool(name="pt", bufs=2, space="PSUM") as ppt,
        ):
            # ---- constants ----
            ident = const.tile([MQ, MQ], DT.float32, tag="ident")
            make_identity(nc, ident)
            iota_i = const.tile([MQ, 1], DT.int32, tag="iotai")
            nc.gpsimd.iota(iota_i, pattern=[[0, 1]], base=0, channel_multiplier=1)
            iota_f = const.tile([MQ, 1], DT.float32, tag="iotaf")
            nc.gpsimd.tensor_copy(iota_f, iota_i)
            two_col = const.tile([MQ, 1], DT.float32, tag="twocol")
            nc.gpsimd.memset(two_col, TEMPERATURE)
            oneh_s = const.tile([1, NQ * N_WAY], DT.float32, tag="oneh")
            nc.sync.dma_start(out=oneh_s, in_=oneh_d[:])

            # ---- bulk loads: support fully, queries in chunks ----
            sf = supp.tile(sup_shape, mm_dt, tag="sf")
            nc.sync.dma_start(
                out=sf, in_=sup_d[:].rearrange(
                    "p (a b c d) -> p a b c d",
                    a=sup_shape[1], b=sup_shape[2], c=sup_shape[3]))
            qf = qin.tile(qry_shape, mm_dt, tag="qf")
            qchunks = [(0, 10), (10, 19), (19, 29), (29, 38)]
            qr = qry_d[:].rearrange(
                "p (q a b m) -> p q a b m",
                q=NQ, a=qry_shape[2], b=qry_shape[3])
            for lo, hi in qchunks:
                nc.sync.dma_start(out=qf[:, lo:hi], in_=qr[:, lo:hi])

            prow = outp.tile([1, NQ, N_WAY], DT.float32, tag="prow")

            # ---- per-query main loop ----
            for q in range(NQ):
                # S'[m, w, s] into 5 psum banks, one per class
                s_sb = sbp.tile([MQ, N_WAY, MS], DT.float16, tag="ssb")
                smax8 = sm.tile([MQ, 8], DT.float16, tag="smax8")
                nc.gpsimd.memset(smax8[:, N_WAY:], NEG16)
                for w in range(N_WAY):
                    if mode == "fp8":
                        pw = pps.tile([MQP, MS], DT.float32, tag="sbank")
                        for t in range(NT):
                            nc.tensor.matmul(
                                pw, qf[:, q, t], sf[:, t, :, w, 0:MS],
                                start=(t == 0), stop=(t == NT - 1),
                                perf_mode=PM.DoubleRow,
                            )
                        nc.scalar.activation(
                            s_sb[:, w], pw[0:MQ], AF.Copy, scale=out_scale)
                    else:
                        pw = pps.tile([MQ, MS], DT.float32, tag="sbank")
                        for cc in range(C // 128):
                            nc.tensor.matmul(
                                pw, qf[:, q, cc], sf[:, cc, w],
                                start=(cc == 0), stop=(cc == C // 128 - 1),
                            )
                        nc.scalar.activation(
                            s_sb[:, w], pw, AF.Copy, scale=out_scale)

                # per-class max -> smax8[:, :5] (fp16 scan of s_sb)
                nc.vector.tensor_reduce(
                    smax8[:, 0:N_WAY], s_sb, axis=mybir.AxisListType.X, op=OP.max)
                top8 = sm.tile([MQ, 8], DT.float16, tag="top8")
                nc.vector.max(out=top8, in_=smax8)
                idx8 = sm.tile([MQ, 8], DT.uint32, tag="idx8")
                nc.vector.max_index(idx8, top8, s_sb.rearrange("m w s -> m (w s)"))

                # nd2 = [nearest, top2-diff] as fp32 columns
                nd2 = sm.tile([MQ, 2], DT.float32, tag="nd2")
                nc.gpsimd.tensor_copy(nd2[:, 0:1], idx8[:, 0:1])
                nc.gpsimd.tensor_sub(nd2[:, 1:2], top8[:, 0:1], top8[:, 1:2])

                # transpose to a row pair, then broadcast down partitions
                nd2t = ppt.tile([1, 2 * MQ], DT.float32, tag="tiny")
                nc.tensor.transpose(nd2t[:, 0:MQ], nd2[:, 0:1], ident)
                nc.tensor.transpose(nd2t[:, MQ:], nd2[:, 1:2], ident)
                ndrow = sm.tile([1, 2 * MQ], DT.float32, tag="ndrow")
                nc.scalar.copy(ndrow, nd2t)
                ndb = sm.tile([MQ, 2 * MQ], DT.float32, tag="ndb")
                nc.gpsimd.partition_broadcast(ndb, ndrow)

                # score[m, m'] = (nearest[m']==nearest[m]) * diff[m']
                score = sm.tile([MQ, MQ], DT.float32, tag="score")
                nc.vector.scalar_tensor_tensor(
                    out=score, in0=ndb[:, 0:MQ], scalar=nd2[:, 0:1],
                    in1=ndb[:, MQ:], op0=OP.is_equal, op1=OP.mult,
                )
                stop8 = sm.tile([MQ, 8], DT.float32, tag="stop8")
                nc.vector.max(out=stop8, in_=score)
                sidx8 = sm.tile([MQ, 8], DT.uint32, tag="sidx8")
                nc.vector.max_index(sidx8, stop8, score)
                winf = sm.tile([MQ, 1], DT.float32, tag="winf")
                nc.gpsimd.tensor_copy(winf, sidx8[:, 0:1])
                masks = sm.tile([MQ, 1], DT.float16, tag="masks")
                nc.vector.scalar_tensor_tensor(
                    out=masks, in0=winf, scalar=iota_f, in1=two_col,
                    op0=OP.is_equal, op1=OP.mult,
                )

                # predict[w] = sum_m masks[m] * smax[m, w]
                pred = ppt.tile([1, N_WAY], DT.float32, tag="tiny")
                nc.tensor.matmul(
                    pred, masks, smax8[:, 0:N_WAY], start=True, stop=True)
                nc.scalar.copy(prow[:, q], pred)

            # ---- epilogue: per-query -loss contributions ----
            pmax = outp.tile([1, NQ], DT.float32, tag="pmax")
            nc.vector.tensor_reduce(pmax, prow, axis=mybir.AxisListType.X, op=OP.max)
            tcen = outp.tile([1, NQ, N_WAY], DT.float32, tag="tcen")
            nc.vector.tensor_sub(tcen, prow, pmax.to_broadcast([1, NQ, N_WAY]))
            esum = outp.tile([1, NQ], DT.float32, tag="esum")
            ee = outp.tile([1, NQ, N_WAY], DT.float32, tag="ee")
            nc.scalar.activation(ee, tcen, AF.Exp)
            nc.vector.tensor_reduce(esum, ee, axis=mybir.AxisListType.X, op=OP.add)
            lse = outp.tile([1, NQ], DT.float32, tag="lse")
            nc.scalar.activation(lse, esum, AF.Ln)
            py = outp.tile([1, NQ], DT.float32, tag="py")
            tg = outp.tile([1, NQ, N_WAY], DT.float32, tag="tg")
            nc.vector.tensor_mul(
                tg, tcen, oneh_s.rearrange("o (q w) -> o q w", w=N_WAY))
            nc.vector.tensor_reduce(py, tg, axis=mybir.AxisListType.X, op=OP.add)
            lossv = outp.tile([1, NQ], DT.float32, tag="lossv")
            nc.vector.tensor_sub(lossv, py, lse)
            nc.sync.dma_start(out=loss_d[:], in_=lossv)

    nc.compile()
    return nc


def _normalize(x, axis):
    return x / (np.linalg.norm(x, axis=axis, keepdims=True) + EPS)


def shard_inputs(support_xf, query_xf, query_y, mode="fp8"):
    """Full inputs -> per-core input dicts (8 cores).

    Host-side prep: reshape to descriptor form, L2-normalize over the
    channel dim (same as the reference), then pack into the on-chip
    layout (scaled/padded fp8 for the DoubleRow path).
    """
    support_xf = np.asarray(support_xf, dtype=np.float32)
    query_xf = np.asarray(query_xf, dtype=np.float32)
    query_y = np.asarray(query_y)

    # [b, w, c, M_s] and [b, q, c, M_q], normalized over c
    sup = support_xf.reshape(B, N_WAY, K_SHOT, C, HW)
    sup = np.transpose(sup, (0, 1, 3, 2, 4)).reshape(B, N_WAY, C, MS)
    sup = _normalize(sup, 2)
    qry = query_xf.reshape(B, Q, C, HW)
    qry = _normalize(qry, 2)

    if mode == "fp8":
        sup = (sup * IN_SCALE).astype(ml_dtypes.float8_e4m3)
        qry = (qry * IN_SCALE).astype(ml_dtypes.float8_e4m3)
        supp = np.zeros((B, N_WAY, C_PAD, MSP), dtype=ml_dtypes.float8_e4m3)
        qryp = np.zeros((B, Q, C_PAD, MQP), dtype=ml_dtypes.float8_e4m3)
        supp[:, :, :C, :MS] = sup
        qryp[:, :, :C, :MQ] = qry
        # channel c = t*256 + k2*128 + cp
        # sup dev layout [cp, t, k2, w, s]; qry dev [cp, q, t, k2, m]
        sup_dev = supp.reshape(B, N_WAY, NT, 2, 128, MSP)
        sup_dev = np.transpose(sup_dev, (0, 4, 2, 3, 1, 5))  # b,cp,t,k2,w,s
        qry_dev = qryp.reshape(B, Q, NT, 2, 128, MQP)
        qry_dev = np.transpose(qry_dev, (0, 4, 1, 2, 3, 5))  # b,cp,q,t,k2,m
        np_dt = ml_dtypes.float8_e4m3
    else:
        CC = C // 128
        sup_dev = sup.reshape(B, N_WAY, CC, 128, MS)
        sup_dev = np.transpose(sup_dev, (0, 3, 2, 1, 4))     # b,cp,cc,w,s
        qry_dev = qry.reshape(B, Q, CC, 128, MQ)
        qry_dev = np.transpose(qry_dev, (0, 3, 1, 2, 4))     # b,cp,q,cc,m
        np_dt = np.float32

    in_maps = []
    for core in range(8):
        b = core // 2
        qs = 0 if core % 2 == 0 else Q - NQ  # 0 or 37
        y = query_y[b, qs:qs + NQ].astype(np.int64)
        oneh = np.zeros((NQ, N_WAY), dtype=np.float32)
        oneh[np.arange(NQ), y] = 1.0
        in_maps.append({
            "sup": np.ascontiguousarray(
                sup_dev[b].reshape(128, -1)).astype(np_dt),
            "qry": np.ascontiguousarray(
                qry_dev[b, :, qs:qs + NQ].reshape(128, -1)).astype(np_dt),
            "oneh": oneh.reshape(1, NQ * N_WAY),
        })
    return in_maps


def gather_loss(results):
    """Per-core [1, NQ] -logp rows -> scalar mean loss."""
    total = 0.0
    for core in range(8):
        row = np.asarray(results[core]["loss"]).reshape(NQ)
        take = row if core % 2 == 0 else row[NQ - (Q - NQ):]  # drop overlap
        total += float(take.sum())
    return np.float32(-total / (B * Q))


_CACHED = {}


def kernel(support_xf, support_y, query_xf, query_y):
    key = "nc"
    if key not in _CACHED:
        _CACHED[key] = build_kernel()
    nc = _CACHED[key]
    in_maps = shard_inputs(support_xf, query_xf, query_y)
    res = run_bass_kernel_spmd(nc, in_maps, list(range(8)))
    return gather_loss(res.results)


if __name__ == "__main__":
    rng = np.random.default_rng(0)
    sup = rng.standard_normal((B, 25, C, 10, 10), dtype=np.float32)
    qry = rng.standard_normal((B, Q, C, 10, 10), dtype=np.float32)
    sy = rng.integers(0, N_WAY, (B, 25))
    qy = rng.integers(0, N_WAY, (B, Q))
    print(kernel(sup, sy, qry, qy))


# revision 9
# speedup vs baseline: 1.8860x; 1.8860x over previous
"""DMN4 topk-masking kernel for Trainium2 (8 NeuronCores, Bass/Tile).

Problem: few-shot episodic loss (DMN4). For each (episode b, query q):
  - cosine similarity S[m, g] between 100 query descriptors (m) and
    2500 support descriptors (g = class w * 500 + shot k * 100 + pos p),
    contracting over c=640 channels.
  - per-class max S_max[w, m]; global argmax "nearest[m]"; top-2 class
    diff; mutual-nearest mask; predict[w] = sum_m S_max*mask*2;
    loss = NLL(log_softmax(predict), y), meaned over all b*q.

Sharding: data-parallel over (b, q). 8 cores = 4 episodes x 2 query
halves; each core processes 38 queries of one episode (cores 2k+1
overlap one query which the host drops when gathering).

Implementation notes:
  - Descriptors are L2-normalized on the host, scaled by 16, padded
    from 640 to 768 channels and cast to fp8 e4m3. The PE then runs
    DoubleRow fp8 matmuls (K=256 per pass, 0.5 cycles/column), halving
    tensor-engine time vs fp32r. The 16x16=256 scale is folded back in
    the PSUM->SBUF copy (scale=1/256).
  - S' is staged to SBUF as fp16 (halves DVE scan cost); per-class max,
    global argmax and all comparisons are done on the fp16 copy, which
    keeps the max/find value-matching self-consistent.
  - The mutual-nearest scatter/gather chain is a 100x100 "same-slot"
    comparison matrix: winner[m] = first argmax over m' of
    (nearest[m']==nearest[m]) * diff[m'], mask = winner==m.
  - Glue ops (index casts, top-2 diff, partition broadcast, score and
    mask element-wise ops) run on the otherwise-idle GpSimd engine to
    keep DVE free for the two big scans (reduce + find_index).
"""

import numpy as np
import ml_dtypes

from concourse import bacc, bass, mybir
from concourse.bass_utils import run_bass_kernel_spmd
from concourse.masks import make_identity
from concourse.tile import TileContext

DT = mybir.dt
AF = mybir.ActivationFunctionType
OP = mybir.AluOpType
PM = mybir.MatmulPerfMode

N_WAY = 5
K_SHOT = 5
TEMPERATURE = 2.0
EPS = 1e-8
B, Q, C, HW = 4, 75, 640, 100
MQ = HW            # query descriptors per query image
MS = K_SHOT * HW   # support descriptors per class
NS = N_WAY * MS    # 2500 support descriptors total
NQ = 38            # queries per core (2 cores x 38 covers 75 with 1 overlap)
IN_SCALE = 16.0    # host-side scale before fp8 rounding (avoids denormals)

# fp8 DoubleRow path: channels padded to 768 = 3 pairs of 128
C_PAD = 768
NT = C_PAD // 256  # 3 DoubleRow passes of K=256
MQP = 128          # query descriptors padded (dual-fp8 ldweights alignment)
MSP = 512          # per-class support cols padded (dual-fp8 ifmap alignment)
NEG16 = -60000.0   # below any similarity, representable in fp16


def build_kernel(mode="fp8"):
    """One SPMD program; every core runs the same 38-query episode slice."""
    nc = bacc.Bacc("TRN2", target_bir_lowering=False, debug=False, num_devices=8)

    if mode == "fp8":
        mm_dt = DT.float8e4
        sup_shape = [128, NT, 2, N_WAY, MSP]  # [cp, t, k2, w, s]
        qry_shape = [128, NQ, NT, 2, MQP]     # [cp, q, t, k2, m]
        out_scale = 1.0 / (IN_SCALE * IN_SCALE)
    else:  # fp32r fallback
        mm_dt = DT.float32r
        CC = C // 128
        sup_shape = [128, CC, N_WAY, MS]
        qry_shape = [128, NQ, CC, MQ]
        out_scale = 1.0

    sup_d = nc.declare_dram_parameter(
        "sup", [128, int(np.prod(sup_shape[1:]))], mm_dt, False)
    qry_d = nc.declare_dram_parameter(
        "qry", [128, int(np.prod(qry_shape[1:]))], mm_dt, False)
    oneh_d = nc.declare_dram_parameter("oneh", [1, NQ * N_WAY], DT.float32, False)
    loss_d = nc.declare_dram_parameter("loss", [1, NQ], DT.float32, True)

    with TileContext(nc) as tc:
        with (
            tc.tile_pool(name="const", bufs=1) as const,
            tc.tile_pool(name="sup", bufs=1) as supp,
            tc.tile_pool(name="qin", bufs=1) as qin,
            tc.tile_pool(name="sb", bufs=4) as sbp,
            tc.tile_pool(name="small", bufs=5) as sm,
            tc.tile_pool(name="out", bufs=1) as outp,
            tc.tile_pool(name="ps", bufs=6, space="PSUM") as pps,
            tc.tile_pool(name="pt", bufs=2, space="PSUM") as ppt,
        ):
            # ---- constants ----
            ident = const.tile([MQ, MQ], DT.float32, tag="ident")
            make_identity(nc, ident)
            iota_i = const.tile([MQ, 1], DT.int32, tag="iotai")
            nc.gpsimd.iota(iota_i, pattern=[[0, 1]], base=0, channel_multiplier=1)
            iota_f = const.tile([MQ, 1], DT.float32, tag="iotaf")
            nc.gpsimd.tensor_copy(iota_f, iota_i)
            two_col = const.tile([MQ, 1], DT.float32, tag="twocol")
            nc.gpsimd.memset(two_col, TEMPERATURE)
            oneh_s = const.tile([1, NQ * N_WAY], DT.float32, tag="oneh")
            nc.sync.dma_start(out=oneh_s, in_=oneh_d[:])

            # ---- bulk loads: support fully, queries in chunks ----
            sf = supp.tile(sup_shape, mm_dt, tag="sf")
            nc.sync.dma_start(
                out=sf, in_=sup_d[:].rearrange(
                    "p (a b c d) -> p a b c d",
                    a=sup_shape[1], b=sup_shape[2], c=sup_shape[3]))
            qf = qin.tile(qry_shape, mm_dt, tag="qf")
            qchunks = [(0, 10), (10, 19), (19, 29), (29, 38)]
            qr = qry_d[:].rearrange(
                "p (q a b m) -> p q a b m",
                q=NQ, a=qry_shape[2], b=qry_shape[3])
            for lo, hi in qchunks:
                nc.sync.dma_start(out=qf[:, lo:hi], in_=qr[:, lo:hi])

            prow = outp.tile([1, NQ, N_WAY], DT.float32, tag="prow")

            # ---- per-query main loop, software-pipelined ----
            # Stage A(q): matmuls -> psum->sbuf copies -> class max, global
            #   argmax, top-2 diff, transpose + broadcast of (nearest, diff).
            # Stage B(q): 100x100 mutual-nearest score, winner mask, predict.
            # Emission interleaves A(i) with B(i-D) so each engine's in-order
            # queue always has the next query's bulk work queued ahead of the
            # cross-engine decision chain of the previous one (the chain
            # latency is hidden behind the big DVE scans of later queries).
            state = {}

            def stage_a(q):
                s_sb = sbp.tile([MQ, N_WAY, MS], DT.float16, tag="ssb")
                smax8 = sm.tile([MQ, 8], DT.float16, tag="smax8")
                for w in range(N_WAY):
                    if mode == "fp8":
                        pw = pps.tile([MQP, MS], DT.float32, tag="sbank")
                        for t in range(NT):
                            nc.tensor.matmul(
                                pw, qf[:, q, t], sf[:, t, :, w, 0:MS],
                                start=(t == 0), stop=(t == NT - 1),
                                perf_mode=PM.DoubleRow,
                            )
                        nc.scalar.activation(
                            s_sb[:, w], pw[0:MQ], AF.Copy, scale=out_scale)
                    else:
                        pw = pps.tile([MQ, MS], DT.float32, tag="sbank")
                        for cc in range(C // 128):
                            nc.tensor.matmul(
                                pw, qf[:, q, cc], sf[:, cc, w],
                                start=(cc == 0), stop=(cc == C // 128 - 1),
                            )
                        nc.scalar.activation(
                            s_sb[:, w], pw, AF.Copy, scale=out_scale)

                # per-class max -> smax8[:, :5] (fp16 scan of s_sb)
                nc.vector.tensor_reduce(
                    smax8[:, 0:N_WAY], s_sb, axis=mybir.AxisListType.X, op=OP.max)
                nc.vector.memset(smax8[:, N_WAY:], NEG16)
                top8 = sm.tile([MQ, 8], DT.float16, tag="top8")
                nc.vector.max(out=top8, in_=smax8)
                idx8 = sm.tile([MQ, 8], DT.uint32, tag="idx8")
                nc.vector.max_index(idx8, top8, s_sb.rearrange("m w s -> m (w s)"))

                # nd2 = [nearest, top2-diff] as fp32 columns
                nd2 = sm.tile([MQ, 2], DT.float32, tag="nd2")
                nc.vector.tensor_copy(nd2[:, 0:1], idx8[:, 0:1])
                nc.vector.tensor_sub(nd2[:, 1:2], top8[:, 0:1], top8[:, 1:2])

                # transpose to a row pair, then broadcast down partitions
                nd2t = ppt.tile([1, 2 * MQ], DT.float32, tag="tiny")
                nc.tensor.transpose(nd2t[:, 0:MQ], nd2[:, 0:1], ident)
                nc.tensor.transpose(nd2t[:, MQ:], nd2[:, 1:2], ident)
                ndrow = sm.tile([1, 2 * MQ], DT.float32, tag="ndrow")
                nc.scalar.copy(ndrow, nd2t)
                ndb = sm.tile([MQ, 2 * MQ], DT.float32, tag="ndb")
                nc.gpsimd.partition_broadcast(ndb, ndrow)
                state[q] = (smax8, nd2, ndb)

            def stage_b(q):
                smax8, nd2, ndb = state.pop(q)
                # score[m, m'] = (nearest[m']==nearest[m]) * diff[m']
                score = sm.tile([MQ, MQ], DT.float32, tag="score")
                nc.vector.scalar_tensor_tensor(
                    out=score, in0=ndb[:, 0:MQ], scalar=nd2[:, 0:1],
                    in1=ndb[:, MQ:], op0=OP.is_equal, op1=OP.mult,
                )
                stop8 = sm.tile([MQ, 8], DT.float32, tag="stop8")
                nc.vector.max(out=stop8, in_=score)
                sidx8 = sm.tile([MQ, 8], DT.uint32, tag="sidx8")
                nc.vector.max_index(sidx8, stop8, score)
                winf = sm.tile([MQ, 1], DT.float32, tag="winf")
                nc.vector.tensor_copy(winf, sidx8[:, 0:1])
                masks = sm.tile([MQ, 1], DT.float16, tag="masks")
                nc.vector.scalar_tensor_tensor(
                    out=masks, in0=winf, scalar=iota_f, in1=two_col,
                    op0=OP.is_equal, op1=OP.mult,
                )
                # predict[w] = sum_m masks[m] * smax[m, w]
                pred = ppt.tile([1, N_WAY], DT.float32, tag="tiny")
                nc.tensor.matmul(
                    pred, masks, smax8[:, 0:N_WAY], start=True, stop=True)
                nc.scalar.copy(prow[:, q], pred)

            DEPTH = 2
            for i in range(NQ + DEPTH):
                if i < NQ:
                    stage_a(i)
                if i >= DEPTH:
                    stage_b(i - DEPTH)

            # ---- epilogue: per-query -loss contributions ----
            pmax = outp.tile([1, NQ], DT.float32, tag="pmax")
            nc.vector.tensor_reduce(pmax, prow, axis=mybir.AxisListType.X, op=OP.max)
            tcen = outp.tile([1, NQ, N_WAY], DT.float32, tag="tcen")
            nc.vector.tensor_sub(tcen, prow, pmax.to_broadcast([1, NQ, N_WAY]))
            esum = outp.tile([1, NQ], DT.float32, tag="esum")
            ee = outp.tile([1, NQ, N_WAY], DT.float32, tag="ee")
            nc.scalar.activation(ee, tcen, AF.Exp)
            nc.vector.tensor_reduce(esum, ee, axis=mybir.AxisListType.X, op=OP.add)
            lse = outp.tile([1, NQ], DT.float32, tag="lse")
            nc.scalar.activation(lse, esum, AF.Ln)
            py = outp.tile([1, NQ], DT.float32, tag="py")
            tg = outp.tile([1, NQ, N_WAY], DT.float32, tag="tg")
            nc.vector.tensor_mul(
                tg, tcen, oneh_s.rearrange("o (q w) -> o q w", w=N_WAY))
            nc.vector.tensor_reduce(py, tg, axis=mybir.AxisListType.X, op=OP.add)
            lossv = outp.tile([1, NQ], DT.float32, tag="lossv")
            nc.vector.tensor_sub(lossv, py, lse)
            nc.sync.dma_start(out=loss_d[:], in_=lossv)

    nc.compile()
    return nc


def _normalize(x, axis):
    return x / (np.linalg.norm(x, axis=axis, keepdims=True) + EPS)


def shard_inputs(support_xf, query_xf, query_y, mode="fp8"):
    """Full inputs -> per-core input dicts (8 cores).

    Host-side prep: reshape to descriptor form, L2-normalize over the
    channel dim (same as the reference), then pack into the on-chip
    layout (scaled/padded fp8 for the DoubleRow path).
    """
    support_xf = np.asarray(support_xf, dtype=np.float32)
    query_xf = np.asarray(query_xf, dtype=np.float32)
    query_y = np.asarray(query_y)

    # [b, w, c, M_s] and [b, q, c, M_q], normalized over c
    sup = support_xf.reshape(B, N_WAY, K_SHOT, C, HW)
    sup = np.transpose(sup, (0, 1, 3, 2, 4)).reshape(B, N_WAY, C, MS)
    sup = _normalize(sup, 2)
    qry = query_xf.reshape(B, Q, C, HW)
    qry = _normalize(qry, 2)

    if mode == "fp8":
        sup = (sup * IN_SCALE).astype(ml_dtypes.float8_e4m3)
        qry = (qry * IN_SCALE).astype(ml_dtypes.float8_e4m3)
        supp = np.zeros((B, N_WAY, C_PAD, MSP), dtype=ml_dtypes.float8_e4m3)
        qryp = np.zeros((B, Q, C_PAD, MQP), dtype=ml_dtypes.float8_e4m3)
        supp[:, :, :C, :MS] = sup
        qryp[:, :, :C, :MQ] = qry
        # channel c = t*256 + k2*128 + cp
        # sup dev layout [cp, t, k2, w, s]; qry dev [cp, q, t, k2, m]
        sup_dev = supp.reshape(B, N_WAY, NT, 2, 128, MSP)
        sup_dev = np.transpose(sup_dev, (0, 4, 2, 3, 1, 5))  # b,cp,t,k2,w,s
        qry_dev = qryp.reshape(B, Q, NT, 2, 128, MQP)
        qry_dev = np.transpose(qry_dev, (0, 4, 1, 2, 3, 5))  # b,cp,q,t,k2,m
        np_dt = ml_dtypes.float8_e4m3
    else:
        CC = C // 128
        sup_dev = sup.reshape(B, N_WAY, CC, 128, MS)
        sup_dev = np.transpose(sup_dev, (0, 3, 2, 1, 4))     # b,cp,cc,w,s
        qry_dev = qry.reshape(B, Q, CC, 128, MQ)
        qry_dev = np.transpose(qry_dev, (0, 3, 1, 2, 4))     # b,cp,q,cc,m
        np_dt = np.float32

    in_maps = []
    for core in range(8):
        b = core // 2
        qs = 0 if core % 2 == 0 else Q - NQ  # 0 or 37
        y = query_y[b, qs:qs + NQ].astype(np.int64)
        oneh = np.zeros((NQ, N_WAY), dtype=np.float32)
        oneh[np.arange(NQ), y] = 1.0
        in_maps.append({
            "sup": np.ascontiguousarray(
                sup_dev[b].reshape(128, -1)).astype(np_dt),
            "qry": np.ascontiguousarray(
                qry_dev[b, :, qs:qs + NQ].reshape(128, -1)).astype(np_dt),
            "oneh": oneh.reshape(1, NQ * N_WAY),
        })
    return in_maps


def gather_loss(results):
    """Per-core [1, NQ] -logp rows -> scalar mean loss."""
    total = 0.0
    for core in range(8):
        row = np.asarray(results[core]["loss"]).reshape(NQ)
        take = row if core % 2 == 0 else row[NQ - (Q - NQ):]  # drop overlap
        total += float(take.sum())
    return np.float32(-total / (B * Q))


_CACHED = {}


def kernel(support_xf, support_y, query_xf, query_y):
    key = "nc"
    if key not in _CACHED:
        _CACHED[key] = build_kernel()
    nc = _CACHED[key]
    in_maps = shard_inputs(support_xf, query_xf, query_y)
    res = run_bass_kernel_spmd(nc, in_maps, list(range(8)))
    return gather_loss(res.results)


if __name__ == "__main__":
    rng = np.random.default_rng(0)
    sup = rng.standard_normal((B, 25, C, 10, 10), dtype=np.float32)
    qry = rng.standard_normal((B, Q, C, 10, 10), dtype=np.float32)
    sy = rng.integers(0, N_WAY, (B, 25))
    qy = rng.integers(0, N_WAY, (B, Q))
    print(kernel(sup, sy, qry, qy))
